# revision 2
# baseline (speedup 1.0000x reference)
"""Trainium2 Bass kernel for nn_Bridge_61538291417809 (moe_routing / SDM block).

Computation (see reference):
  x   = LayerNorm(h) * ln_scale + ln_bias
  xb  = x @ w_in.T                         [N, DB]
  g   = silu(xb @ sdm_gate.T)              [N, I]
  idx = top_k(|g|, 128)  (== top_k by raw gate logit; see note below)
  gu  = g[idx] * (xb @ sdm_up.T)[idx]
  rb  = scatter(gu) @ sdm_down.T           [N, DB]
  out = h + (rb @ w_out.T) * tanh(gate_small)

Sharding (8 cores):
  - stage 1 (LN folded into proj_in): output-sharded over DB, then AllGather
  - gate/up/down: tensor-parallel over I (padded to a multiple of 1024)
  - top-k: local per-core top-128 by raw logit, AllToAll candidate exchange,
    each core finds the exact global 128th-largest for its 64-token slice of
    each 512-token chunk, AllGather of thresholds, mask by (logit >= t)
  - down produces partial sums over I-shards -> ReduceScatter over tokens
  - w_out + gated residual on each core's own tokens; host reassembles.

Precision:
  - proj_in / gate / up matmuls run in fp32r (TF32-grade operand rounding,
    fp32 accumulate, full bf16-rate throughput). Measured logit error
    ~2.2e-4 std keeps top-k selection within ~1.4e-2 output rel err under
    a worst-case (randn) gate_small; the staged reference uses zeros.
  - down / w_out matmuls and the ReduceScatter run bf16 (value-only error).

Top-k by raw logit: top-128 of |silu(z)| equals top-128 of z as long as the
128th largest silu value exceeds max_{z<0} |silu(z)| = 0.2785; here the
threshold is ~2.9, so selection by raw logit matches selection by |silu|.
"""

import os
import sys

sys.path.insert(0, "/opt/trn_rl_repo")

import numpy as np
import ml_dtypes

BF16 = ml_dtypes.bfloat16

NCORES = 8


def full_cfg():
    return dict(NT=4096, DS=2048, DB=5120, I=13824, TOPK=128, TCH=512)


def _derived(cfg):
    d = dict(cfg)
    d["NCH"] = cfg["NT"] // cfg["TCH"]          # token chunks
    d["OWN"] = cfg["TCH"] // NCORES             # owned tokens per core per chunk
    d["ILOC"] = -(-cfg["I"] // NCORES // 128) * 128   # padded I shard
    d["ESH"] = cfg["DB"] // NCORES              # stage-1 output shard
    d["KT1"] = cfg["DS"] // 128
    d["MT1"] = d["ESH"] // 128
    d["KT2"] = cfg["DB"] // 128
    d["CT"] = d["ILOC"] // 128
    d["EC"] = cfg["DB"] // 512
    d["WN"] = cfg["DS"] // 512
    d["R"] = cfg["TOPK"] // 8                   # max8 rounds
    assert cfg["TCH"] % 128 == 0 and cfg["TCH"] == 512
    assert d["ESH"] % 128 == 0 and cfg["DS"] % 512 == 0 and cfg["DB"] % 512 == 0
    return d


def build_program(cfg, single_core=False):
    """single_core=True: build a 1-device variant with collectives replaced
    by local DMA copies — wrong results, same structure; for TimelineSim."""
    import concourse.bacc as bacc
    import concourse.mybir as mybir
    import concourse.tile as tile
    from concourse.masks import make_identity

    dt = mybir.dt
    d = _derived(cfg)
    NT, DS, DB, TOPK, TCH = cfg["NT"], cfg["DS"], cfg["DB"], cfg["TOPK"], cfg["TCH"]
    NCH, OWN, ILOC, ESH = d["NCH"], d["OWN"], d["ILOC"], d["ESH"]
    KT1, MT1, KT2, CT, EC, WN, R = (
        d["KT1"], d["MT1"], d["KT2"], d["CT"], d["EC"], d["WN"], d["R"])
    RG = [list(range(NCORES))]
    KH = KT2 // 2                               # weight half-panel k-tiles

    nc = bacc.Bacc("TRN2", target_bir_lowering=False, debug=False,
                   num_devices=1 if single_core else NCORES)

    def collective(kind, op, ins, outs):
        if not single_core:
            nc.gpsimd.collective_compute(kind, op, replica_groups=RG,
                                         ins=ins, outs=outs)
            return
        # local stand-in with roughly equivalent DMA traffic
        ia, oa = ins[0], outs[0]
        if kind == "AllGather":
            n = ia.shape[0]
            for r in range(NCORES):
                nc.sync.dma_start(out=oa[r * n:(r + 1) * n], in_=ia)
        elif kind == "AllToAll":
            nc.sync.dma_start(out=oa, in_=ia)
        elif kind == "ReduceScatter":
            n = oa.shape[0]
            nc.sync.dma_start(out=oa, in_=ia[:n])

    def din(name, shape, dty):
        return nc.dram_tensor(name, shape, dty, kind="ExternalInput")

    def dint(name, shape, dty, shared=False):
        if shared:
            return nc.dram_tensor(name, shape, dty, addr_space="Shared")
        return nc.dram_tensor(name, shape, dty)

    bf = dt.bfloat16
    f32 = dt.float32
    f32r = dt.float32r

    hT = din("hT", [DS, NT], f32r)
    W1r = din("W1r", [DS, ESH], f32r)
    r1c = din("r1c", [ESH], f32)
    c1c = din("c1c", [ESH], f32)
    rho = din("rho", [NT], f32)
    rhomu = din("rhomu", [NT], f32)
    # pre-swizzled on host: [p, ct, kt, mm] so a panel load is contiguous
    gTr = din("gTr", [128, CT, KT2, 128], f32r)
    uTr = din("uTr", [128, CT, KT2, 128], f32r)
    dTm = din("dTm", [ILOC, DB], bf)
    woT = din("woT", [DB, DS], bf)
    h_own = din("h_own", [NCH * OWN, DS], f32)
    out = nc.dram_tensor("out", [NCH * OWN, DS], f32, kind="ExternalOutput")

    xsh = dint("xsh", [ESH, NT], f32r)
    x_full = dint("x_full", [DB, NT], f32r, shared=True)
    cand_d = [dint(f"cand{c}", [TCH, TOPK], f32) for c in range(NCH)]
    cA2A_d = [dint(f"cA2A{c}", [TCH, TOPK], f32) for c in range(NCH)]
    tloc_d = [dint(f"tloc{c}", [OWN], f32) for c in range(NCH)]
    tAG_d = [dint(f"tAG{c}", [TCH], f32, shared=True) for c in range(NCH)]
    prb_d = [dint(f"prb{c}", [TCH, DB], bf) for c in range(NCH)]
    gaT_d = [dint(f"gaT{c}", [TCH // 128, 128, ILOC], f32) for c in range(NCH)]
    rb_own = dint("rb_own", [NCH * OWN, DB], bf)

    with tile.TileContext(nc) as tc:
        from contextlib import ExitStack
        with ExitStack() as octx:
            const = octx.enter_context(tc.tile_pool(name="const", bufs=1))
            psum = octx.enter_context(tc.tile_pool(name="psum", bufs=2, space="PSUM"))
            psum1 = octx.enter_context(tc.tile_pool(name="psum1", bufs=1, space="PSUM"))
            ident = const.tile([128, 128], f32)
            make_identity(nc, ident)
            ident_bf = const.tile([128, 128], bf)
            make_identity(nc, ident_bf)

            # ---------------- stage 1: xT = fold_ln(proj_in) ----------------
            with ExitStack() as s1:
                s1c = s1.enter_context(tc.tile_pool(name="s1c", bufs=1))
                s1x = s1.enter_context(tc.tile_pool(name="s1x", bufs=2))
                s1t = s1.enter_context(tc.tile_pool(name="s1t", bufs=3))

                W1_sb = s1c.tile([128, KT1, ESH], f32r)
                nc.sync.dma_start(out=W1_sb[:], in_=W1r.ap().rearrange("(k p) m -> p k m", p=128))
                r1_sb = s1c.tile([128, MT1], f32)
                c1_sb = s1c.tile([128, MT1], f32)
                nc.sync.dma_start(out=r1_sb[:], in_=r1c.ap().rearrange("(m p) -> p m", p=128))
                nc.sync.dma_start(out=c1_sb[:], in_=c1c.ap().rearrange("(m p) -> p m", p=128))

                for ntc in range(NT // TCH):
                    tsl = slice(ntc * TCH, (ntc + 1) * TCH)
                    hh = s1x.tile([128, KT1, TCH], f32r, tag="hh")
                    nc.sync.dma_start(out=hh[:], in_=hT.ap()[:, tsl].rearrange("(k p) n -> p k n", p=128))
                    rho_sb = s1t.tile([1, TCH], f32, tag="rho")
                    rmu_sb = s1t.tile([1, TCH], f32, tag="rmu")
                    nc.sync.dma_start(out=rho_sb[:], in_=rho.ap()[tsl].unsqueeze(0))
                    nc.sync.dma_start(out=rmu_sb[:], in_=rhomu.ap()[tsl].unsqueeze(0))
                    rho_bc = s1t.tile([128, TCH], f32, tag="rhob")
                    rmu_bc = s1t.tile([128, TCH], f32, tag="rmub")
                    nc.gpsimd.partition_broadcast(rho_bc[:], rho_sb[:1, :])
                    nc.gpsimd.partition_broadcast(rmu_bc[:], rmu_sb[:1, :])

                    for mt in range(MT1):
                        ps = psum.tile([128, TCH], f32, tag="psA")
                        msl = slice(mt * 128, (mt + 1) * 128)
                        for kt in range(KT1):
                            nc.tensor.matmul(ps[:], W1_sb[:, kt, msl], hh[:, kt],
                                             start=(kt == 0), stop=(kt == KT1 - 1))
                        t1 = s1t.tile([128, TCH], f32, tag="t1")
                        x32 = s1t.tile([128, TCH], f32, tag="x32")
                        nc.vector.tensor_scalar(t1[:], rmu_bc[:], r1_sb[:, mt:mt + 1], None,
                                                op0=mybir.AluOpType.mult)
                        nc.vector.tensor_tensor(x32[:], ps[:], rho_bc[:],
                                                op=mybir.AluOpType.mult)
                        nc.vector.tensor_sub(x32[:], x32[:], t1[:])
                        nc.vector.tensor_scalar_add(x32[:], x32[:], c1_sb[:, mt:mt + 1])
                        # cast-DMA f32 -> f32r (bit-identical), gpsimd only
                        nc.gpsimd.dma_start(out=xsh.ap()[msl, tsl], in_=x32[:])

            collective("AllGather", mybir.AluOpType.bypass,
                       [xsh.ap()], [x_full.ap()])

            # ---------------- stage 2: gate/up, topk, down -------------------
            # x chunk fully resident (f32r); gate logits bounce through DRAM
            # in token-major layout so chunks pipeline freely.
            with ExitStack() as s2:
                s2x = s2.enter_context(tc.tile_pool(name="s2x", bufs=1))
                s2w = s2.enter_context(tc.tile_pool(name="s2w", bufs=3))
                s2gu = s2.enter_context(tc.tile_pool(name="s2gu", bufs=2))
                s2t = s2.enter_context(tc.tile_pool(name="s2t", bufs=1))
                s2tk = s2.enter_context(tc.tile_pool(name="s2tk", bufs=1))
                s2m = s2.enter_context(tc.tile_pool(name="s2m", bufs=2))
                s2d = s2.enter_context(tc.tile_pool(name="s2d", bufs=6))
                s2o = s2.enter_context(tc.tile_pool(name="s2o", bufs=2))

                for c in range(NCH):
                    tsl = slice(c * TCH, (c + 1) * TCH)
                    xh_c = s2x.tile([128, KT2, TCH], f32r, tag="xh")
                    nc.sync.dma_start(out=xh_c[:], in_=x_full.ap()[:, tsl].rearrange("(k p) n -> p k n", p=128))

                    guv_all = s2gu.tile([128, CT, TCH], bf, tag="guv")

                    for ct in range(CT):
                        csl = slice(ct * 128, (ct + 1) * 128)
                        gpA = s2w.tile([128, KH, 128], f32r, tag="gp")
                        gpB = s2w.tile([128, KH, 128], f32r, tag="gp")
                        upA = s2w.tile([128, KH, 128], f32r, tag="up")
                        upB = s2w.tile([128, KH, 128], f32r, tag="up")
                        nc.sync.dma_start(out=gpA[:], in_=gTr.ap()[:, ct, :KH])
                        nc.sync.dma_start(out=gpB[:], in_=gTr.ap()[:, ct, KH:])
                        nc.sync.dma_start(out=upA[:], in_=uTr.ap()[:, ct, :KH])
                        nc.sync.dma_start(out=upB[:], in_=uTr.ap()[:, ct, KH:])
                        psg = psum.tile([128, TCH], f32, tag="psA")
                        for kt in range(KT2):
                            wt = gpA if kt < KH else gpB
                            nc.tensor.matmul(psg[:], wt[:, kt % KH], xh_c[:, kt],
                                             start=(kt == 0), stop=(kt == KT2 - 1))
                        psu = psum.tile([128, TCH], f32, tag="psB")
                        for kt in range(KT2):
                            wt = upA if kt < KH else upB
                            nc.tensor.matmul(psu[:], wt[:, kt % KH], xh_c[:, kt],
                                             start=(kt == 0), stop=(kt == KT2 - 1))
                        # evict: sgf = logits fp32 in SBUF
                        sgf = s2t.tile([128, TCH], f32, tag="sgf")
                        nc.vector.tensor_copy(sgf[:], psg[:])
                        for tg in range(TCH // 128):
                            pst = psum.tile([128, 128], f32, tag="psT")
                            nc.tensor.transpose(
                                pst[:], sgf[:, tg * 128:(tg + 1) * 128], ident[:])
                            stg = s2m.tile([128, 128], f32, tag="stg")
                            nc.vector.tensor_copy(stg[:], pst[:])
                            nc.sync.dma_start(out=gaT_d[c].ap()[tg, :, csl], in_=stg[:])
                        # guv = silu(z)*u = z*sigmoid(z)*u
                        sg = s2t.tile([128, TCH], f32, tag="sg")
                        nc.scalar.activation(sg[:], psg[:],
                                             mybir.ActivationFunctionType.Sigmoid)
                        nc.vector.tensor_mul(sg[:], sg[:], sgf[:])
                        nc.vector.tensor_mul(guv_all[:, ct], sg[:], psu[:])

                    # local top-128 per token (by raw logit), per 128-token group
                    for tg in range(TCH // 128):
                        scrA = s2tk.tile([128, CT * 128], f32, tag="tkA")
                        nc.sync.dma_start(out=scrA[:], in_=gaT_d[c].ap()[tg])
                        cand_sb = s2m.tile([128, TOPK], f32, tag="cand")
                        for r in range(R):
                            nc.vector.max(cand_sb[:, r * 8:(r + 1) * 8], scrA[:])
                            nc.vector.match_replace(scrA[:], cand_sb[:, r * 8:(r + 1) * 8],
                                                    scrA[:], -1e30)
                        nc.sync.dma_start(out=cand_d[c].ap()[tg * 128:(tg + 1) * 128, :],
                                          in_=cand_sb[:])

                    collective("AllToAll", mybir.AluOpType.bypass,
                               [cand_d[c].ap()], [cA2A_d[c].ap()])

                    # exact global threshold for own OWN tokens
                    thA = s2tk.tile([OWN, NCORES * TOPK], f32, tag="thA")
                    nc.sync.dma_start(
                        out=thA[:],
                        in_=cA2A_d[c].ap().rearrange("(r j) k -> j r k", j=OWN))
                    tc8 = s2m.tile([OWN, 8], f32, tag="tc8")
                    for r in range(R):
                        nc.vector.max(tc8[:], thA[:])
                        nc.vector.match_replace(thA[:], tc8[:], thA[:], -1e30)
                    nc.sync.dma_start(out=tloc_d[c].ap(), in_=tc8[:, 7:8])

                    collective("AllGather", mybir.AluOpType.bypass,
                               [tloc_d[c].ap()], [tAG_d[c].ap()])

                    # mask: m01T = (logit >= t) token-major, transpose back,
                    # multiply into guv
                    t_cols = s2m.tile([128, TCH // 128], f32, tag="tcols")
                    nc.sync.dma_start(out=t_cols[:],
                                      in_=tAG_d[c].ap().rearrange("(g p) -> p g", p=128))
                    for tg in range(TCH // 128):
                        gaTm = s2tk.tile([128, CT * 128], f32, tag="tkA")
                        nc.sync.dma_start(out=gaTm[:], in_=gaT_d[c].ap()[tg])
                        m01T = s2tk.tile([128, CT * 128], bf, tag="m01T")
                        nc.vector.tensor_scalar(m01T[:], gaTm[:],
                                                t_cols[:, tg:tg + 1], None,
                                                op0=mybir.AluOpType.is_ge)
                        gsl = slice(tg * 128, (tg + 1) * 128)
                        for ct in range(CT):
                            pstm = psum.tile([128, 128], bf, tag="psT")
                            nc.tensor.transpose(
                                pstm[:], m01T[:, ct * 128:(ct + 1) * 128], ident_bf[:])
                            nc.vector.tensor_mul(guv_all[:, ct, gsl],
                                                 guv_all[:, ct, gsl], pstm[:])

                    # down: partial r_big for this chunk (token groups in
                    # pairs so each dT tile load serves two PSUM banks)
                    for ec in range(DB // 512):
                        esl = slice(ec * 512, (ec + 1) * 512)
                        for tgp in range(TCH // 256):
                            g0 = slice(tgp * 256, tgp * 256 + 128)
                            g1 = slice(tgp * 256 + 128, tgp * 256 + 256)
                            psd0 = psum1.tile([128, 512], f32, tag="psD0")
                            psd1 = psum1.tile([128, 512], f32, tag="psD1")
                            for ct in range(CT):
                                dpt = s2d.tile([128, 512], bf, tag="dp")
                                nc.sync.dma_start(
                                    out=dpt[:],
                                    in_=dTm.ap()[ct * 128:(ct + 1) * 128, esl])
                                nc.tensor.matmul(psd0[:], guv_all[:, ct, g0], dpt[:],
                                                 start=(ct == 0), stop=(ct == CT - 1))
                                nc.tensor.matmul(psd1[:], guv_all[:, ct, g1], dpt[:],
                                                 start=(ct == 0), stop=(ct == CT - 1))
                            for gi, psd in ((g0, psd0), (g1, psd1)):
                                ot = s2o.tile([128, 512], bf, tag="prbo")
                                nc.scalar.copy(ot[:], psd[:])
                                nc.sync.dma_start(out=prb_d[c].ap()[gi, esl], in_=ot[:])

                    collective("ReduceScatter", mybir.AluOpType.add,
                               [prb_d[c].ap()],
                               [rb_own.ap()[c * OWN:(c + 1) * OWN, :]])

            # ---------------- stage 3: w_out + residual ----------------------
            with ExitStack() as s3:
                s3r = s3.enter_context(tc.tile_pool(name="s3r", bufs=2))
                s3rt = s3.enter_context(tc.tile_pool(name="s3rt", bufs=1))
                s3w = s3.enter_context(tc.tile_pool(name="s3w", bufs=2))
                s3o = s3.enter_context(tc.tile_pool(name="s3o", bufs=3))
                NTOK = NCH * OWN
                MT4 = NTOK // 128
                rbT_all = s3rt.tile([128, MT4, KT2, 128], bf)
                for mt4 in range(MT4):
                    rsl = slice(mt4 * 128, (mt4 + 1) * 128)
                    rb_sb = s3r.tile([128, DB], bf, tag="rb")
                    nc.sync.dma_start(out=rb_sb[:], in_=rb_own.ap()[rsl, :])
                    for kt in range(KT2):
                        pst = psum.tile([128, 128], bf, tag="psT")
                        nc.tensor.transpose(pst[:], rb_sb[:, kt * 128:(kt + 1) * 128], ident_bf[:])
                        nc.vector.tensor_copy(rbT_all[:, mt4, kt], pst[:])
                for wn in range(WN):
                    wsl = slice(wn * 512, (wn + 1) * 512)
                    wo_p = s3w.tile([128, KT2, 512], bf, tag="wo")
                    nc.sync.dma_start(out=wo_p[:], in_=woT.ap()[:, wsl].rearrange("(k p) n -> p k n", p=128))
                    for mt4 in range(MT4):
                        rsl = slice(mt4 * 128, (mt4 + 1) * 128)
                        psw = psum.tile([128, 512], f32, tag="psA")
                        for kt in range(KT2):
                            nc.tensor.matmul(psw[:], rbT_all[:, mt4, kt], wo_p[:, kt],
                                             start=(kt == 0), stop=(kt == KT2 - 1))
                        hres = s3o.tile([128, 512], f32, tag="hres")
                        nc.sync.dma_start(out=hres[:], in_=h_own.ap()[rsl, wsl])
                        oo = s3o.tile([128, 512], f32, tag="oo")
                        nc.vector.tensor_add(oo[:], psw[:], hres[:])
                        nc.sync.dma_start(out=out.ap()[rsl, wsl], in_=oo[:])

    nc.compile()
    return nc


# ----------------------------- host side ---------------------------------

def host_prep(inputs, cfg):
    d = _derived(cfg)
    NT, DS, DB, I, TCH = cfg["NT"], cfg["DS"], cfg["DB"], cfg["I"], cfg["TCH"]
    NCH, OWN, ILOC, ESH = d["NCH"], d["OWN"], d["ILOC"], d["ESH"]

    h = np.asarray(inputs["h"], np.float32).reshape(NT, DS)
    ln_scale = np.asarray(inputs["ln_scale"], np.float32)
    ln_bias = np.asarray(inputs["ln_bias"], np.float32)
    w_in = np.asarray(inputs["w_in"], np.float32)
    w_out = np.asarray(inputs["w_out"], np.float32)
    gate_small = np.asarray(inputs["gate_small"], np.float32)
    sdm_gate = np.asarray(inputs["sdm_gate"], np.float32)
    sdm_up = np.asarray(inputs["sdm_up"], np.float32)
    sdm_down = np.asarray(inputs["sdm_down"], np.float32)

    mu = h.mean(axis=1, dtype=np.float64)
    var = np.square(h - mu[:, None].astype(np.float32)).mean(axis=1, dtype=np.float64)
    rstd = (1.0 / np.sqrt(var + 1e-5)).astype(np.float32)
    mu = mu.astype(np.float32)

    hT = np.ascontiguousarray(h.T)                      # [DS, NT] f32

    W1 = np.ascontiguousarray((w_in * ln_scale[None, :]).T)  # [DS, DB] f32
    r1 = (w_in * ln_scale[None, :]).sum(axis=1).astype(np.float32)   # [DB]
    c1 = (w_in @ ln_bias).astype(np.float32)                          # [DB]

    gateT = np.ascontiguousarray(sdm_gate.T)            # [DB, I]
    upT = np.ascontiguousarray(sdm_up.T)                # [DB, I]
    downT = np.ascontiguousarray(sdm_down.T)            # [I, DB]

    tg = np.tanh(gate_small).astype(np.float32)
    woT2 = np.ascontiguousarray((w_out * tg[:, None]).T)  # [DB, DS]
    woT2_bf = woT2.astype(BF16)

    iloc_raw = I // NCORES
    KT2 = DB // 128
    CT = ILOC // 128

    def swz(arr_db_iloc):
        # [DB, ILOC] -> [p, ct, kt, mm] with k = kt*128+p, m = ct*128+mm
        t = arr_db_iloc.reshape(KT2, 128, CT, 128)
        return np.ascontiguousarray(t.transpose(1, 2, 0, 3))

    in_maps = []
    own_idx = []
    for m in range(NCORES):
        gsh = np.zeros((DB, ILOC), np.float32)
        ush = np.zeros((DB, ILOC), np.float32)
        dsh = np.zeros((ILOC, DB), BF16)
        isl = slice(m * iloc_raw, (m + 1) * iloc_raw)
        gsh[:, :iloc_raw] = gateT[:, isl]
        ush[:, :iloc_raw] = upT[:, isl]
        dsh[:iloc_raw, :] = downT[isl, :].astype(BF16)
        gsh, ush = swz(gsh), swz(ush)

        esl = slice(m * ESH, (m + 1) * ESH)
        idx_m = np.array([c * TCH + m * OWN + j for c in range(NCH) for j in range(OWN)])
        own_idx.append(idx_m)

        in_maps.append({
            "hT": hT,
            "W1r": np.ascontiguousarray(W1[:, esl]),
            "r1c": np.ascontiguousarray(r1[esl]),
            "c1c": np.ascontiguousarray(c1[esl]),
            "rho": rstd,
            "rhomu": (rstd * mu).astype(np.float32),
            "gTr": gsh,
            "uTr": ush,
            "dTm": dsh,
            "woT": woT2_bf,
            "h_own": np.ascontiguousarray(h[idx_m]),
        })
    return in_maps, own_idx


_PROG_CACHE = {}


def _get_program(cfg):
    key = tuple(sorted(cfg.items()))
    if key not in _PROG_CACHE:
        _PROG_CACHE[key] = build_program(cfg)
    return _PROG_CACHE[key]


def run_on_hw(inputs, cfg, trace=False):
    from concourse.bass_utils import run_bass_kernel_spmd
    nc = _get_program(cfg)
    in_maps, own_idx = host_prep(inputs, cfg)
    res = run_bass_kernel_spmd(nc, in_maps, list(range(NCORES)), trace=trace)
    d = _derived(cfg)
    NT, DS = cfg["NT"], cfg["DS"]
    out = np.empty((NT, DS), np.float32)
    for m in range(NCORES):
        out[own_idx[m]] = res.results[m]["out"]
    return out, res


def kernel(**inputs):
    cfg = full_cfg()
    out, _ = run_on_hw(inputs, cfg)
    B, S = 2, 2048
    return out.reshape(B, S, cfg["DS"]).astype(np.float32)


if __name__ == "__main__":
    pass


# revision 3
# speedup vs baseline: 1.3626x; 1.3626x over previous
"""Trainium2 Bass kernel for nn_Bridge_61538291417809 (moe_routing / SDM block).

Computation (see reference):
  x   = LayerNorm(h) * ln_scale + ln_bias
  xb  = x @ w_in.T                         [N, DB]
  g   = silu(xb @ sdm_gate.T)              [N, I]
  idx = top_k(|g|, 128)  (== top_k by raw gate logit; see note below)
  gu  = g[idx] * (xb @ sdm_up.T)[idx]
  rb  = scatter(gu) @ sdm_down.T           [N, DB]
  out = h + (rb @ w_out.T) * tanh(gate_small)

Sharding (8 cores):
  - stage 1 (LN folded into proj_in): output-sharded over DB, AllGather per
    512-token chunk so stage 2 starts before stage 1 finishes
  - gate/up/down: tensor-parallel over I (padded to a multiple of 1024)
  - top-k: local per-core top-48 by raw logit (48 >= max plausible per-core
    share of a global top-128; P[miss] ~ 1e-6 over all tokens), AllToAll
    candidate exchange, each core finds the exact global 128th-largest for
    its 64-token slice of each chunk, AllGather of thresholds, mask by
    (logit >= t)
  - down produces partial sums over I-shards -> ReduceScatter over tokens
  - w_out + gated residual on each core's own tokens; host reassembles.

Pipelining: the chunk loop issues gate/up(c) then mask+down(c-1), so the
topk -> A2A -> threshold -> AG chain of chunk c runs on DVE/TOPSP while the
PE is busy with chunk c+1's gate/up matmuls.

Precision:
  - proj_in / gate / up matmuls run in fp32r (TF32-grade operand rounding,
    fp32 accumulate, full bf16-rate throughput). Measured logit error
    ~2.2e-4 std keeps top-k selection within ~1.4e-2 output rel err under
    a worst-case (randn) gate_small; the staged reference uses zeros.
  - down / w_out path runs fp16 (10-bit mantissa) end to end, including the
    ReduceScatter.
"""

import os
import sys

sys.path.insert(0, "/opt/trn_rl_repo")

import numpy as np
import ml_dtypes

BF16 = ml_dtypes.bfloat16

NCORES = 8
TOPC = 48               # local top-k candidates sent per core


def full_cfg():
    return dict(NT=4096, DS=2048, DB=5120, I=13824, TOPK=128, TCH=512)


def _derived(cfg):
    d = dict(cfg)
    d["NCH"] = cfg["NT"] // cfg["TCH"]          # token chunks
    d["OWN"] = cfg["TCH"] // NCORES             # owned tokens per core per chunk
    d["ILOC"] = -(-cfg["I"] // NCORES // 128) * 128   # padded I shard
    d["ESH"] = cfg["DB"] // NCORES              # stage-1 output shard
    d["KT1"] = cfg["DS"] // 128
    d["MT1"] = d["ESH"] // 128
    d["KT2"] = cfg["DB"] // 128
    d["CT"] = d["ILOC"] // 128
    d["EC"] = cfg["DB"] // 512
    d["WN"] = cfg["DS"] // 512
    d["R"] = cfg["TOPK"] // 8                   # threshold max8 rounds
    d["RC"] = TOPC // 8                         # candidate max8 rounds
    assert cfg["TCH"] % 128 == 0 and cfg["TCH"] == 512
    assert d["ESH"] % 128 == 0 and cfg["DS"] % 512 == 0 and cfg["DB"] % 512 == 0
    return d


def build_program(cfg, single_core=False):
    import concourse.bacc as bacc
    import concourse.mybir as mybir
    import concourse.tile as tile
    from concourse.masks import make_identity

    dt = mybir.dt
    d = _derived(cfg)
    NT, DS, DB, TOPK, TCH = cfg["NT"], cfg["DS"], cfg["DB"], cfg["TOPK"], cfg["TCH"]
    NCH, OWN, ILOC, ESH = d["NCH"], d["OWN"], d["ILOC"], d["ESH"]
    KT1, MT1, KT2, CT, EC, WN, R, RC = (
        d["KT1"], d["MT1"], d["KT2"], d["CT"], d["EC"], d["WN"], d["R"], d["RC"])
    RG = [list(range(NCORES))]
    KH = KT2 // 2                               # weight half-panel k-tiles

    nc = bacc.Bacc("TRN2", target_bir_lowering=False, debug=False,
                   num_devices=1 if single_core else NCORES)

    def collective(kind, op, ins, outs):
        if not single_core:
            nc.gpsimd.collective_compute(kind, op, replica_groups=RG,
                                         ins=ins, outs=outs)
            return
        ia, oa = ins[0], outs[0]
        if kind == "AllGather":
            n = ia.shape[0]
            for r in range(NCORES):
                nc.sync.dma_start(out=oa[r * n:(r + 1) * n], in_=ia)
        elif kind == "AllToAll":
            nc.sync.dma_start(out=oa, in_=ia)
        elif kind == "ReduceScatter":
            n = oa.shape[0]
            nc.sync.dma_start(out=oa, in_=ia[:n])

    def din(name, shape, dty):
        return nc.dram_tensor(name, shape, dty, kind="ExternalInput")

    def dint(name, shape, dty, shared=False):
        if shared:
            return nc.dram_tensor(name, shape, dty, addr_space="Shared")
        return nc.dram_tensor(name, shape, dty)

    bf = dt.bfloat16
    f16 = dt.float16
    f32 = dt.float32
    f32r = dt.float32r

    hT = din("hT", [DS, NT], f32r)
    W1r = din("W1r", [DS, ESH], f32r)
    r1c = din("r1c", [ESH], f32)
    c1c = din("c1c", [ESH], f32)
    rho = din("rho", [NT], f32)
    rhomu = din("rhomu", [NT], f32)
    # pre-swizzled on host: [p, ct, kt, mm] so a panel load is contiguous
    gTr = din("gTr", [128, CT, KT2, 128], f32r)
    uTr = din("uTr", [128, CT, KT2, 128], f32r)
    dTm = din("dTm", [ILOC, DB], f16)
    woT = din("woT", [DB, DS], f16)
    h_own = din("h_own", [NCH * OWN, DS], f32)
    out = nc.dram_tensor("out", [NCH * OWN, DS], f32, kind="ExternalOutput")

    xsh_d = [dint(f"xsh{c}", [ESH, TCH], f32r) for c in range(NCH)]
    xfull_d = [dint(f"xfull{c}", [DB, TCH], f32r, shared=True) for c in range(NCH)]
    cand_d = [dint(f"cand{c}", [TCH, TOPC], f32) for c in range(NCH)]
    cA2A_d = [dint(f"cA2A{c}", [TCH, TOPC], f32) for c in range(NCH)]
    tloc_d = [dint(f"tloc{c}", [OWN], f32) for c in range(NCH)]
    tAG_d = [dint(f"tAG{c}", [TCH], f32, shared=True) for c in range(NCH)]
    prb_d = [dint(f"prb{c}", [TCH, DB], f16) for c in range(NCH)]
    gaT_d = [dint(f"gaT{c}", [TCH // 128, 128, ILOC], f32) for c in range(NCH)]
    rb_own = dint("rb_own", [NCH * OWN, DB], f16)

    with tile.TileContext(nc) as tc:
        from contextlib import ExitStack
        with ExitStack() as octx:
            const = octx.enter_context(tc.tile_pool(name="const", bufs=1))
            psum = octx.enter_context(tc.tile_pool(name="psum", bufs=2, space="PSUM"))
            psum1 = octx.enter_context(tc.tile_pool(name="psum1", bufs=1, space="PSUM"))
            ident = const.tile([128, 128], f32)
            make_identity(nc, ident)
            ident_f16 = const.tile([128, 128], f16)
            make_identity(nc, ident_f16)

            # ---------------- stage 1: xT = fold_ln(proj_in) ----------------
            with ExitStack() as s1:
                s1c = s1.enter_context(tc.tile_pool(name="s1c", bufs=1))
                s1x = s1.enter_context(tc.tile_pool(name="s1x", bufs=2))
                s1t = s1.enter_context(tc.tile_pool(name="s1t", bufs=3))

                W1_sb = s1c.tile([128, KT1, ESH], f32r)
                nc.sync.dma_start(out=W1_sb[:], in_=W1r.ap().rearrange("(k p) m -> p k m", p=128))
                r1_sb = s1c.tile([128, MT1], f32)
                c1_sb = s1c.tile([128, MT1], f32)
                nc.sync.dma_start(out=r1_sb[:], in_=r1c.ap().rearrange("(m p) -> p m", p=128))
                nc.sync.dma_start(out=c1_sb[:], in_=c1c.ap().rearrange("(m p) -> p m", p=128))

                for ntc in range(NCH):
                    tsl = slice(ntc * TCH, (ntc + 1) * TCH)
                    hh = s1x.tile([128, KT1, TCH], f32r, tag="hh")
                    nc.sync.dma_start(out=hh[:], in_=hT.ap()[:, tsl].rearrange("(k p) n -> p k n", p=128))
                    rho_sb = s1t.tile([1, TCH], f32, tag="rho")
                    rmu_sb = s1t.tile([1, TCH], f32, tag="rmu")
                    nc.sync.dma_start(out=rho_sb[:], in_=rho.ap()[tsl].unsqueeze(0))
                    nc.sync.dma_start(out=rmu_sb[:], in_=rhomu.ap()[tsl].unsqueeze(0))
                    rho_bc = s1t.tile([128, TCH], f32, tag="rhob")
                    rmu_bc = s1t.tile([128, TCH], f32, tag="rmub")
                    nc.gpsimd.partition_broadcast(rho_bc[:], rho_sb[:1, :])
                    nc.gpsimd.partition_broadcast(rmu_bc[:], rmu_sb[:1, :])

                    for mt in range(MT1):
                        ps = psum.tile([128, TCH], f32, tag="psA")
                        msl = slice(mt * 128, (mt + 1) * 128)
                        for kt in range(KT1):
                            nc.tensor.matmul(ps[:], W1_sb[:, kt, msl], hh[:, kt],
                                             start=(kt == 0), stop=(kt == KT1 - 1))
                        t1 = s1t.tile([128, TCH], f32, tag="t1")
                        x32 = s1t.tile([128, TCH], f32, tag="x32")
                        nc.vector.tensor_scalar(t1[:], rmu_bc[:], r1_sb[:, mt:mt + 1], None,
                                                op0=mybir.AluOpType.mult)
                        nc.vector.tensor_tensor(x32[:], ps[:], rho_bc[:],
                                                op=mybir.AluOpType.mult)
                        nc.vector.tensor_sub(x32[:], x32[:], t1[:])
                        nc.vector.tensor_scalar_add(x32[:], x32[:], c1_sb[:, mt:mt + 1])
                        # cast-DMA f32 -> f32r (bit-identical), gpsimd only
                        nc.gpsimd.dma_start(out=xsh_d[ntc].ap()[msl, :], in_=x32[:])

                    collective("AllGather", mybir.AluOpType.bypass,
                               [xsh_d[ntc].ap()], [xfull_d[ntc].ap()])

            # ---------------- stage 2: gate/up, topk, down -------------------
            with ExitStack() as s2:
                s2x = s2.enter_context(tc.tile_pool(name="s2x", bufs=1))
                s2w = s2.enter_context(tc.tile_pool(name="s2w", bufs=3))
                s2gu = s2.enter_context(tc.tile_pool(name="s2gu", bufs=2))
                s2t = s2.enter_context(tc.tile_pool(name="s2t", bufs=1))
                s2tk = s2.enter_context(tc.tile_pool(name="s2tk", bufs=1))
                s2m = s2.enter_context(tc.tile_pool(name="s2m", bufs=2))
                s2d = s2.enter_context(tc.tile_pool(name="s2d", bufs=6))
                s2o = s2.enter_context(tc.tile_pool(name="s2o", bufs=2))

                guv_tiles = {}

                def chunk_gateup(c):
                    xh_c = s2x.tile([128, KT2, TCH], f32r, tag="xh", name=f"xh{c}")
                    nc.sync.dma_start(out=xh_c[:], in_=xfull_d[c].ap().rearrange("(k p) n -> p k n", p=128))

                    guv_all = s2gu.tile([128, CT, TCH], f16, tag="guv", name=f"guv{c}")
                    guv_tiles[c] = guv_all

                    for ct in range(CT):
                        csl = slice(ct * 128, (ct + 1) * 128)
                        gpA = s2w.tile([128, KH, 128], f32r, tag="gp", name=f"gpA{c}_{ct}")
                        gpB = s2w.tile([128, KH, 128], f32r, tag="gp", name=f"gpB{c}_{ct}")
                        upA = s2w.tile([128, KH, 128], f32r, tag="up", name=f"upA{c}_{ct}")
                        upB = s2w.tile([128, KH, 128], f32r, tag="up", name=f"upB{c}_{ct}")
                        nc.sync.dma_start(out=gpA[:], in_=gTr.ap()[:, ct, :KH])
                        nc.sync.dma_start(out=gpB[:], in_=gTr.ap()[:, ct, KH:])
                        nc.sync.dma_start(out=upA[:], in_=uTr.ap()[:, ct, :KH])
                        nc.sync.dma_start(out=upB[:], in_=uTr.ap()[:, ct, KH:])
                        psg = psum.tile([128, TCH], f32, tag="psA", name=f"psg{c}_{ct}")
                        for kt in range(KT2):
                            wt = gpA if kt < KH else gpB
                            nc.tensor.matmul(psg[:], wt[:, kt % KH], xh_c[:, kt],
                                             start=(kt == 0), stop=(kt == KT2 - 1))
                        psu = psum.tile([128, TCH], f32, tag="psB", name=f"psu{c}_{ct}")
                        for kt in range(KT2):
                            wt = upA if kt < KH else upB
                            nc.tensor.matmul(psu[:], wt[:, kt % KH], xh_c[:, kt],
                                             start=(kt == 0), stop=(kt == KT2 - 1))
                        # evict: sgf = logits fp32 in SBUF; token-major copy to DRAM
                        sgf = s2t.tile([128, TCH], f32, tag="sgf", name=f"sgf{c}_{ct}")
                        nc.vector.tensor_copy(sgf[:], psg[:])
                        for tg in range(TCH // 128):
                            pst = psum.tile([128, 128], f32, tag="psT", name=f"pst{c}_{ct}_{tg}")
                            nc.tensor.transpose(
                                pst[:], sgf[:, tg * 128:(tg + 1) * 128], ident[:])
                            stg = s2m.tile([128, 128], f32, tag="stg", name=f"stg{c}_{ct}_{tg}")
                            nc.vector.tensor_copy(stg[:], pst[:])
                            nc.sync.dma_start(out=gaT_d[c].ap()[tg, :, csl], in_=stg[:])
                        # guv = silu(z)*u = z*sigmoid(z)*u
                        sg = s2t.tile([128, TCH], f32, tag="sg", name=f"sg{c}_{ct}")
                        nc.scalar.activation(sg[:], psg[:],
                                             mybir.ActivationFunctionType.Sigmoid)
                        nc.vector.tensor_mul(sg[:], sg[:], sgf[:])
                        nc.vector.tensor_mul(guv_all[:, ct], sg[:], psu[:])

                    # local top-TOPC per token (by raw logit), per 128-token group
                    for tg in range(TCH // 128):
                        scrA = s2tk.tile([128, CT * 128], f32, tag="tkS", name=f"scr{c}_{tg}")
                        nc.sync.dma_start(out=scrA[:], in_=gaT_d[c].ap()[tg])
                        cand_sb = s2m.tile([128, TOPC], f32, tag="cand", name=f"cnd{c}_{tg}")
                        for r in range(RC):
                            nc.vector.max(cand_sb[:, r * 8:(r + 1) * 8], scrA[:])
                            nc.vector.match_replace(scrA[:], cand_sb[:, r * 8:(r + 1) * 8],
                                                    scrA[:], -1e30)
                        nc.sync.dma_start(out=cand_d[c].ap()[tg * 128:(tg + 1) * 128, :],
                                          in_=cand_sb[:])

                    collective("AllToAll", mybir.AluOpType.bypass,
                               [cand_d[c].ap()], [cA2A_d[c].ap()])

                    # exact global threshold for own OWN tokens
                    thA = s2tk.tile([OWN, NCORES * TOPC], f32, tag="thA", name=f"thA{c}")
                    nc.sync.dma_start(
                        out=thA[:],
                        in_=cA2A_d[c].ap().rearrange("(r j) k -> j r k", j=OWN))
                    tc8 = s2m.tile([OWN, 8], f32, tag="tc8", name=f"tc8{c}")
                    for r in range(R):
                        nc.vector.max(tc8[:], thA[:])
                        nc.vector.match_replace(thA[:], tc8[:], thA[:], -1e30)
                    nc.sync.dma_start(out=tloc_d[c].ap(), in_=tc8[:, 7:8])

                    collective("AllGather", mybir.AluOpType.bypass,
                               [tloc_d[c].ap()], [tAG_d[c].ap()])

                def chunk_maskdown(c):
                    guv_all = guv_tiles.pop(c)
                    # mask: m01T = (logit >= t) token-major, transpose back,
                    # multiply into guv
                    t_cols = s2m.tile([128, TCH // 128], f32, tag="tcols", name=f"tcl{c}")
                    nc.sync.dma_start(out=t_cols[:],
                                      in_=tAG_d[c].ap().rearrange("(g p) -> p g", p=128))
                    for tg in range(TCH // 128):
                        gaTm = s2tk.tile([128, CT * 128], f32, tag="tkM", name=f"gam{c}_{tg}")
                        nc.sync.dma_start(out=gaTm[:], in_=gaT_d[c].ap()[tg])
                        m01T = s2tk.tile([128, CT * 128], f16, tag="m01T", name=f"m01{c}_{tg}")
                        nc.vector.tensor_scalar(m01T[:], gaTm[:],
                                                t_cols[:, tg:tg + 1], None,
                                                op0=mybir.AluOpType.is_ge)
                        gsl = slice(tg * 128, (tg + 1) * 128)
                        for ct in range(CT):
                            pstm = psum.tile([128, 128], f16, tag="psT", name=f"psm{c}_{tg}_{ct}")
                            nc.tensor.transpose(
                                pstm[:], m01T[:, ct * 128:(ct + 1) * 128], ident_f16[:])
                            nc.vector.tensor_mul(guv_all[:, ct, gsl],
                                                 guv_all[:, ct, gsl], pstm[:])

                    # down: partial r_big for this chunk
                    for ec in range(DB // 512):
                        esl = slice(ec * 512, (ec + 1) * 512)
                        for tgp in range(TCH // 256):
                            g0 = slice(tgp * 256, tgp * 256 + 128)
                            g1 = slice(tgp * 256 + 128, tgp * 256 + 256)
                            psd0 = psum1.tile([128, 512], f32, tag="psD0", name=f"psd0_{c}_{ec}_{tgp}")
                            psd1 = psum1.tile([128, 512], f32, tag="psD1", name=f"psd1_{c}_{ec}_{tgp}")
                            for ct in range(CT):
                                dpt = s2d.tile([128, 512], f16, tag="dp", name=f"dp{c}_{ec}_{tgp}_{ct}")
                                nc.sync.dma_start(
                                    out=dpt[:],
                                    in_=dTm.ap()[ct * 128:(ct + 1) * 128, esl])
                                nc.tensor.matmul(psd0[:], guv_all[:, ct, g0], dpt[:],
                                                 start=(ct == 0), stop=(ct == CT - 1))
                                nc.tensor.matmul(psd1[:], guv_all[:, ct, g1], dpt[:],
                                                 start=(ct == 0), stop=(ct == CT - 1))
                            for gi, psd in ((g0, psd0), (g1, psd1)):
                                ot = s2o.tile([128, 512], f16, tag="prbo", name=f"ot{c}_{ec}_{tgp}_{gi.start}")
                                nc.scalar.copy(ot[:], psd[:])
                                nc.sync.dma_start(out=prb_d[c].ap()[gi, esl], in_=ot[:])

                    collective("ReduceScatter", mybir.AluOpType.add,
                               [prb_d[c].ap()],
                               [rb_own.ap()[c * OWN:(c + 1) * OWN, :]])

                for c in range(NCH):
                    chunk_gateup(c)
                    if c >= 1:
                        chunk_maskdown(c - 1)
                chunk_maskdown(NCH - 1)

            # ---------------- stage 3: w_out + residual ----------------------
            with ExitStack() as s3:
                s3r = s3.enter_context(tc.tile_pool(name="s3r", bufs=2))
                s3rt = s3.enter_context(tc.tile_pool(name="s3rt", bufs=1))
                s3w = s3.enter_context(tc.tile_pool(name="s3w", bufs=2))
                s3o = s3.enter_context(tc.tile_pool(name="s3o", bufs=3))
                NTOK = NCH * OWN
                MT4 = NTOK // 128
                rbT_all = s3rt.tile([128, MT4, KT2, 128], f16)
                for mt4 in range(MT4):
                    rsl = slice(mt4 * 128, (mt4 + 1) * 128)
                    rb_sb = s3r.tile([128, DB], f16, tag="rb", name=f"rb{mt4}")
                    nc.sync.dma_start(out=rb_sb[:], in_=rb_own.ap()[rsl, :])
                    for kt in range(KT2):
                        pst = psum.tile([128, 128], f16, tag="psT", name=f"ps3_{mt4}_{kt}")
                        nc.tensor.transpose(pst[:], rb_sb[:, kt * 128:(kt + 1) * 128], ident_f16[:])
                        nc.vector.tensor_copy(rbT_all[:, mt4, kt], pst[:])
                for wn in range(WN):
                    wsl = slice(wn * 512, (wn + 1) * 512)
                    wo_p = s3w.tile([128, KT2, 512], f16, tag="wo", name=f"wo{wn}")
                    nc.sync.dma_start(out=wo_p[:], in_=woT.ap()[:, wsl].rearrange("(k p) n -> p k n", p=128))
                    for mt4 in range(MT4):
                        rsl = slice(mt4 * 128, (mt4 + 1) * 128)
                        psw = psum.tile([128, 512], f32, tag="psA", name=f"psw{wn}_{mt4}")
                        for kt in range(KT2):
                            nc.tensor.matmul(psw[:], rbT_all[:, mt4, kt], wo_p[:, kt],
                                             start=(kt == 0), stop=(kt == KT2 - 1))
                        hres = s3o.tile([128, 512], f32, tag="hres", name=f"hr{wn}_{mt4}")
                        nc.sync.dma_start(out=hres[:], in_=h_own.ap()[rsl, wsl])
                        oo = s3o.tile([128, 512], f32, tag="oo", name=f"oo{wn}_{mt4}")
                        nc.vector.tensor_add(oo[:], psw[:], hres[:])
                        nc.sync.dma_start(out=out.ap()[rsl, wsl], in_=oo[:])

    nc.compile()
    return nc


# ----------------------------- host side ---------------------------------

def host_prep(inputs, cfg):
    d = _derived(cfg)
    NT, DS, DB, I, TCH = cfg["NT"], cfg["DS"], cfg["DB"], cfg["I"], cfg["TCH"]
    NCH, OWN, ILOC, ESH = d["NCH"], d["OWN"], d["ILOC"], d["ESH"]

    h = np.asarray(inputs["h"], np.float32).reshape(NT, DS)
    ln_scale = np.asarray(inputs["ln_scale"], np.float32)
    ln_bias = np.asarray(inputs["ln_bias"], np.float32)
    w_in = np.asarray(inputs["w_in"], np.float32)
    w_out = np.asarray(inputs["w_out"], np.float32)
    gate_small = np.asarray(inputs["gate_small"], np.float32)
    sdm_gate = np.asarray(inputs["sdm_gate"], np.float32)
    sdm_up = np.asarray(inputs["sdm_up"], np.float32)
    sdm_down = np.asarray(inputs["sdm_down"], np.float32)

    mu = h.mean(axis=1, dtype=np.float64)
    var = np.square(h - mu[:, None].astype(np.float32)).mean(axis=1, dtype=np.float64)
    rstd = (1.0 / np.sqrt(var + 1e-5)).astype(np.float32)
    mu = mu.astype(np.float32)

    hT = np.ascontiguousarray(h.T)                      # [DS, NT] f32

    W1 = np.ascontiguousarray((w_in * ln_scale[None, :]).T)  # [DS, DB] f32
    r1 = (w_in * ln_scale[None, :]).sum(axis=1).astype(np.float32)   # [DB]
    c1 = (w_in @ ln_bias).astype(np.float32)                          # [DB]

    gateT = np.ascontiguousarray(sdm_gate.T)            # [DB, I]
    upT = np.ascontiguousarray(sdm_up.T)                # [DB, I]
    downT = np.ascontiguousarray(sdm_down.T)            # [I, DB]

    tg = np.tanh(gate_small).astype(np.float32)
    woT2 = np.ascontiguousarray((w_out * tg[:, None]).T)  # [DB, DS]
    woT2_f16 = woT2.astype(np.float16)

    iloc_raw = I // NCORES
    KT2 = DB // 128
    CT = ILOC // 128

    def swz(arr_db_iloc):
        # [DB, ILOC] -> [p, ct, kt, mm] with k = kt*128+p, m = ct*128+mm
        t = arr_db_iloc.reshape(KT2, 128, CT, 128)
        return np.ascontiguousarray(t.transpose(1, 2, 0, 3))

    in_maps = []
    own_idx = []
    for m in range(NCORES):
        gsh = np.zeros((DB, ILOC), np.float32)
        ush = np.zeros((DB, ILOC), np.float32)
        dsh = np.zeros((ILOC, DB), np.float16)
        isl = slice(m * iloc_raw, (m + 1) * iloc_raw)
        gsh[:, :iloc_raw] = gateT[:, isl]
        ush[:, :iloc_raw] = upT[:, isl]
        dsh[:iloc_raw, :] = downT[isl, :].astype(np.float16)
        gsh, ush = swz(gsh), swz(ush)

        esl = slice(m * ESH, (m + 1) * ESH)
        idx_m = np.array([c * TCH + m * OWN + j for c in range(NCH) for j in range(OWN)])
        own_idx.append(idx_m)

        in_maps.append({
            "hT": hT,
            "W1r": np.ascontiguousarray(W1[:, esl]),
            "r1c": np.ascontiguousarray(r1[esl]),
            "c1c": np.ascontiguousarray(c1[esl]),
            "rho": rstd,
            "rhomu": (rstd * mu).astype(np.float32),
            "gTr": gsh,
            "uTr": ush,
            "dTm": dsh,
            "woT": woT2_f16,
            "h_own": np.ascontiguousarray(h[idx_m]),
        })
    return in_maps, own_idx


_PROG_CACHE = {}


def _get_program(cfg):
    key = tuple(sorted(cfg.items()))
    if key not in _PROG_CACHE:
        _PROG_CACHE[key] = build_program(cfg)
    return _PROG_CACHE[key]


def run_on_hw(inputs, cfg, trace=False):
    from concourse.bass_utils import run_bass_kernel_spmd
    nc = _get_program(cfg)
    in_maps, own_idx = host_prep(inputs, cfg)
    res = run_bass_kernel_spmd(nc, in_maps, list(range(NCORES)), trace=trace)
    d = _derived(cfg)
    NT, DS = cfg["NT"], cfg["DS"]
    out = np.empty((NT, DS), np.float32)
    for m in range(NCORES):
        out[own_idx[m]] = res.results[m]["out"]
    return out, res


def kernel(**inputs):
    cfg = full_cfg()
    out, _ = run_on_hw(inputs, cfg)
    B, S = 2, 2048
    return out.reshape(B, S, cfg["DS"]).astype(np.float32)


if __name__ == "__main__":
    pass
